# revision 27
# baseline (speedup 1.0000x reference)
"""Trainium2 Bass kernel for nn_CSA_84387517432661 (dense transformer
causal self-attention block: QKV proj + RMSNorm + RoPE + GQA causal SDPA +
output projection).

Sharding: tensor-parallel over heads across 8 NeuronCores. Core c owns
q-heads {2c, 2c+1} and kv-head c//2, computes its heads' attention output
for the full (B, T), then computes a row-parallel PARTIAL output projection
(its 256 y-dims x full 2048 output dims) fused into the attention loop, and
a chunked ReduceScatter(add) sums the partials, leaving each core with its
256-row slice of the final output. The collectives overlap the remaining
attention work instead of serializing at the end.

Numerics: projections run on the PE in fp32r (full-rate fp32, ~1e-4),
attention and output projection in fp16 inputs with fp32 PSUM accumulation.
Softmax is computed without max-subtraction (scores are O(10), exp fits
fp16/fp32 comfortably) and normalization is folded in after attn@v.
"""

import sys

if "/opt/trn_rl_repo" not in sys.path:
    sys.path.insert(0, "/opt/trn_rl_repo")

import math

import numpy as np

NH = 16
NKV = 4
B = 2
D = 2048
HD = 128
N_CORES = 8
ROPE_BASE = 10000.0
ROPE_TSL = 1024
EPS = 1.1920928955078125e-07

_PROGRAM_CACHE = {}


def _rope_tables(T):
    rd = HD
    if T > ROPE_TSL:
        base = ROPE_BASE * (T / ROPE_TSL) ** (rd / (rd - 2))
    else:
        base = ROPE_BASE
    inv_freq = 1.0 / base ** (np.arange(0, rd, 2, dtype=np.float32) / rd)
    t = np.arange(T, dtype=np.float32)
    fr = np.outer(t, inv_freq)  # [T, 64]
    return np.cos(fr), np.sin(fr)


def _build_program(T):
    import concourse.mybir as mybir
    import concourse.tile as tile
    from concourse import bacc

    f32 = mybir.dt.float32
    f16 = mybir.dt.float16
    f32r = mybir.dt.float32r
    AF = mybir.ActivationFunctionType
    MUL = mybir.AluOpType.mult
    ADD = mybir.AluOpType.add

    BT = B * T
    NJ = T // 128        # 128-wide s-chunks per batch
    NQB = T // 512       # 512-wide q superblocks per batch
    NSB = B * NQB        # total superblocks
    NTS = BT // 1024     # 1024-wide col superblocks for the projections
    NDC = D // 128       # 128-row d chunks
    NCH = NSB // 2       # ReduceScatter chunks (1024 cols each)

    nc = bacc.Bacc("TRN2", target_bir_lowering=False, debug=False,
                   num_devices=N_CORES)

    xt_d = nc.declare_dram_parameter("xt", [D, BT], f16, isOutput=False)
    wqkv_d = nc.declare_dram_parameter("wqkv", [128, NDC * 512], f16,
                                       isOutput=False)
    wp_d = nc.declare_dram_parameter("wp", [256, D], f16, isOutput=False)
    cost_d = nc.declare_dram_parameter("cost", [128, BT], f16, isOutput=False)
    sint_d = nc.declare_dram_parameter("sint", [128, BT], f16, isOutput=False)
    gsc_d = nc.declare_dram_parameter("gsc", [128, 3], f32, isOutput=False)
    gbi_d = nc.declare_dram_parameter("gbi", [128, 3], f32, isOutput=False)
    mask_d = nc.declare_dram_parameter("masks", [128, 2048], f16, isOutput=False)
    id_d = nc.declare_dram_parameter("ident", [128, 128], f16, isOutput=False)

    CW = BT // NCH
    partial_ch = [nc.dram_tensor(f"partial{ch}", [D, CW], f16)
                  for ch in range(NCH)]
    rs_out = [nc.dram_tensor(f"rs_out{ch}", [256, CW], f16)
              for ch in range(NCH)]
    outs_d = [nc.declare_dram_parameter(f"out{ch}", [256, CW], f16,
                                        isOutput=True) for ch in range(NCH)]

    with tile.TileContext(nc) as tc:
        with (
            tc.tile_pool(name="consts", bufs=1) as cstp,
            tc.tile_pool(name="wts", bufs=4) as wtsp,
            tc.tile_pool(name="xs", bufs=3) as xsp,
            tc.tile_pool(name="big16", bufs=1) as bigp,
            tc.tile_pool(name="bwork", bufs=3) as bwp,
            tc.tile_pool(name="o16p", bufs=26) as o16p,
            tc.tile_pool(name="cstream", bufs=2) as csp,
            tc.tile_pool(name="a16", bufs=1) as a16p,
            tc.tile_pool(name="ep", bufs=6) as epp,
            tc.tile_pool(name="cnorm", bufs=2) as cnp,
            tc.tile_pool(name="oev", bufs=4) as oevp,
            tc.tile_pool(name="pp", bufs=8, space="PSUM") as pp,
        ):
            # ---- constants ----
            mask_t = cstp.tile([128, 2048], f16, tag="mask")
            nc.sync.dma_start(out=mask_t[:], in_=mask_d.ap())
            id_t = cstp.tile([128, 128], f16, tag="id")
            nc.sync.dma_start(out=id_t[:], in_=id_d.ap())
            g_t = cstp.tile([128, 3], f32, tag="g")
            nc.sync.dma_start(out=g_t[:], in_=gsc_d.ap())
            g2_t = cstp.tile([128, 3], f32, tag="g2")
            nc.sync.dma_start(out=g2_t[:], in_=gbi_d.ap())
            ones16 = cstp.tile([128, 1], f16, tag="ones")
            nc.vector.memset(ones16[:], 1.0)
            ones_row = cstp.tile([1, 128], f32, tag="onesr")
            nc.vector.memset(ones_row[:], 1.0)
            # output-projection weights for this core's 256 y-dims:
            # wp_t[h][yd, od] with yd local to head h
            wp_t = []
            for h in range(2):
                w = cstp.tile([128, D], f16, tag=f"wp{h}", name=f"wp_t{h}")
                nc.sync.dma_start(out=w[:], in_=wp_d.ap()[h * 128:(h + 1) * 128, :])
                wp_t.append(w)

            # ---- phases A+B: token-major QKV projection + norm/rope ----
            # A computes per 128-token chunk tc the tile [128 tok, 512]
            # (cols: q0 0-127 | q1 128-255 | k 256-383 | v 384-511) by using
            # x chunks as the stationary operand. Token-major makes RMSNorm a
            # free-dim reduction (ACT accum_out) and RoPE a column trick; v
            # lands directly in the [keys, hd] layout attention needs.
            # q0/q1/k are then PE-transposed into dims-major a16.
            # wqkv preloaded once: block j holds rows 128j..128j+128 of the
            # [D, 512] lhsT as [128, 512] at cols j*512.
            wqkv_t = cstp.tile([128, NDC * 512], f16, tag="wqkv")
            nc.sync.dma_start(out=wqkv_t[:], in_=wqkv_d.ap())

            NTC = BT // 128
            big_tm = bigp.tile([128, NTC * 512], f16, tag="bigtm")
            a16 = {}
            for name in ("q0", "q1", "k"):
                a16[name] = a16p.tile([128, BT], f16, tag=name,
                                      name="a16_" + name)

            def v16(cc0, cc1):
                # v slice for BT cols [cc0, cc1): [keys, hd] layout
                tc = cc0 // 128
                assert cc1 - cc0 == 128
                return big_tm[:, tc * 512 + 384: tc * 512 + 512]

            pend_tp = []

            def emit_transposes(items):
                for name, tc, o16 in items:
                    tp_ = pp.tile([128, 128], f16, tag="ps", name="tp")
                    nc.tensor.transpose(tp_[:], o16[:], id_t[:])
                    nc.vector.tensor_copy(
                        a16[name][:, tc * 128:(tc + 1) * 128], tp_[:])

            for tcg in range(NTC // 4):
                pa = [pp.tile([128, 512], f32, tag="ps", name=f"pa{i}")
                      for i in range(4)]
                for j in range(NDC):
                    xt_t = xsp.tile([128, 512], f16, tag="x")
                    nc.sync.dma_start(
                        out=xt_t[:],
                        in_=xt_d.ap()[j * 128:(j + 1) * 128,
                                      tcg * 512:(tcg + 1) * 512])
                    for i in range(4):
                        nc.tensor.matmul(
                            pa[i][:],
                            xt_t[:, i * 128:(i + 1) * 128],
                            wqkv_t[:, j * 512:(j + 1) * 512],
                            start=(j == 0), stop=(j == NDC - 1))
                cos_b = csp.tile([128, 512], f16, tag="cosb")
                nc.sync.dma_start(
                    out=cos_b[:],
                    in_=cost_d.ap()[:, tcg * 512:(tcg + 1) * 512])
                sin_b = csp.tile([128, 512], f16, tag="sinb")
                nc.sync.dma_start(
                    out=sin_b[:],
                    in_=sint_d.ap()[:, tcg * 512:(tcg + 1) * 512])
                for i in range(4):
                    tc = tcg * 4 + i
                    bcol = tc * 512
                    nc.scalar.copy(big_tm[:, bcol:bcol + 512], pa[i][:])
                    for idx, name in enumerate(("q0", "q1", "k")):
                        nr = idx * 128
                        sq = bwp.tile([128, 128], f16, tag="sq")
                        ssq = bwp.tile([128, 1], f32, tag="ssq")
                        nc.scalar.activation(sq[:], pa[i][:, nr:nr + 128],
                                             AF.Square, accum_out=ssq[:])
                        rssq = bwp.tile([128, 1], f32, tag="rssq")
                        nc.scalar.activation(
                            rssq[:], ssq[:], AF.Sqrt,
                            scale=g_t[:, idx:idx + 1],
                            bias=g2_t[:, idx:idx + 1])
                        rs = bwp.tile([128, 1], f32, tag="rs")
                        nc.vector.reciprocal(rs[:], rssq[:])
                        xb = big_tm[:, bcol + nr:bcol + nr + 128]
                        cs = cos_b[:, i * 128:(i + 1) * 128]
                        sn = sin_b[:, i * 128:(i + 1) * 128]
                        m1 = bwp.tile([128, 128], f16, tag="m1")
                        nc.vector.scalar_tensor_tensor(
                            m1[:], xb, rs[:], cs, op0=MUL, op1=MUL)
                        m2 = bwp.tile([128, 128], f16, tag="m2")
                        nc.vector.scalar_tensor_tensor(
                            m2[:, 0:64], xb[:, 64:128], rs[:], sn[:, 0:64],
                            op0=MUL, op1=MUL)
                        nc.vector.scalar_tensor_tensor(
                            m2[:, 64:128], xb[:, 0:64], rs[:], sn[:, 64:128],
                            op0=MUL, op1=MUL)
                        o16 = o16p.tile([128, 128], f16, tag="o16")
                        nc.vector.tensor_tensor(o16[:], m1[:], m2[:], op=ADD)
                        pend_tp.append((name, tc, o16))
                if tcg > 0:
                    emit_transposes(pend_tp[:12])
                    pend_tp = pend_tp[12:]
            emit_transposes(pend_tp)

            # ---- phase C: causal attention + fused partial out-projection ----
            inv_sqrt_hd = 1.0 / math.sqrt(HD)
            k16 = a16["k"]
            for b in range(B):
                for Q in range(NQB):
                    njq = 4 * Q + 4
                    yts = [pp.tile([128, 512], f32, tag="ps", name=f"yt{h}")
                           for h in range(2)]
                    zs = [pp.tile([1, 512], f32, tag="ps", name=f"z{h}")
                          for h in range(2)]
                    for j in range(njq):
                        es = []
                        for h in range(2):
                            sc = pp.tile([128, 512], f32, tag="ps")
                            nc.tensor.matmul(
                                sc[:],
                                k16[:, b * T + j * 128: b * T + (j + 1) * 128],
                                a16["q0" if h == 0 else "q1"][
                                    :, b * T + Q * 512: b * T + (Q + 1) * 512],
                                start=True, stop=True)
                            e = epp.tile([128, 512], f16, tag="e")
                            nc.scalar.activation(e[:], sc[:], AF.Exp,
                                                 scale=inv_sqrt_hd)
                            if j >= 4 * Q:
                                r = j - 4 * Q
                                nc.vector.tensor_tensor(
                                    e[:], e[:], mask_t[:, r * 512:(r + 1) * 512],
                                    op=MUL)
                            es.append(e)
                        vslice = v16((b * NJ + j) * 128, (b * NJ + j + 1) * 128)
                        for h in range(2):
                            nc.tensor.matmul(yts[h][:], vslice, es[h][:],
                                             start=(j == 0), stop=(j == njq - 1))
                        for h in range(2):
                            nc.tensor.matmul(zs[h][:], ones16[:], es[h][:],
                                             start=(j == 0), stop=(j == njq - 1))
                    y16s = []
                    for h in range(2):
                        rz = cnp.tile([1, 512], f32, tag="rz")
                        nc.vector.reciprocal(rz[:], zs[h][:])
                        rzb = pp.tile([128, 512], f32, tag="ps", name="rzb")
                        nc.tensor.matmul(rzb[:], ones_row[:], rz[:],
                                         start=True, stop=True)
                        rzb_s = cnp.tile([128, 512], f32, tag="rzbs")
                        nc.scalar.copy(rzb_s[:], rzb[:])
                        y16 = cnp.tile([128, 512], f16, tag="y16",
                                       name=f"y16_{h}")
                        nc.vector.tensor_tensor(y16[:], yts[h][:], rzb_s[:],
                                                op=MUL)
                        y16s.append(y16)

                    # fused row-parallel out-projection for this superblock
                    ci = b * NQB + Q
                    ch, cc0 = ci // 2, (ci % 2) * 512
                    for od in range(NDC):
                        pso = pp.tile([128, 512], f32, tag="ps", name="pso")
                        nc.tensor.matmul(pso[:],
                                         wp_t[0][:, od * 128:(od + 1) * 128],
                                         y16s[0][:], start=True, stop=False)
                        nc.tensor.matmul(pso[:],
                                         wp_t[1][:, od * 128:(od + 1) * 128],
                                         y16s[1][:], start=False, stop=True)
                        po = oevp.tile([128, 512], f16, tag="po")
                        nc.vector.tensor_copy(po[:], pso[:])
                        nc.sync.dma_start(
                            out=partial_ch[ch].ap()[od * 128:(od + 1) * 128,
                                                    cc0:cc0 + 512],
                            in_=po[:])
                    if ci % 2 == 1:
                        nc.gpsimd.collective_compute(
                            "ReduceScatter", ADD,
                            replica_groups=[list(range(N_CORES))],
                            ins=[partial_ch[ch].ap()],
                            outs=[rs_out[ch].ap()])
                        nc.gpsimd.dma_start(out=outs_d[ch].ap(),
                                            in_=rs_out[ch].ap())

    nc.finalize()
    return nc


def _prepare_in_maps(x, Wq, Wk, Wv, Wp, q_gain):
    Bx, T, Dx = x.shape
    assert (Bx, Dx) == (B, D)
    BT = B * T

    x = np.asarray(x, dtype=np.float32)
    Wq = np.asarray(Wq, dtype=np.float32)
    Wk = np.asarray(Wk, dtype=np.float32)
    Wv = np.asarray(Wv, dtype=np.float32)
    Wp = np.asarray(Wp, dtype=np.float32)
    q_gain = np.asarray(q_gain, dtype=np.float32)

    xt_np = np.ascontiguousarray(x.reshape(BT, D).T.astype(np.float16))  # [D, BT]

    cos_, sin_ = _rope_tables(T)  # [T, 64]
    # token-major tables: chunk tc at cols [tc*128, (tc+1)*128), row = token
    # within chunk; [cos|cos] and [sin|-sin] along the 128 head dims.
    NTC = BT // 128
    cc = np.tile(np.concatenate([cos_, cos_], axis=1), (B, 1))  # [BT, 128]
    ss = np.tile(np.concatenate([sin_, -sin_], axis=1), (B, 1))
    cost_np = np.ascontiguousarray(
        cc.reshape(NTC, 128, 128).transpose(1, 0, 2).reshape(128, -1)
        .astype(np.float16))
    sint_np = np.ascontiguousarray(
        ss.reshape(NTC, 128, 128).transpose(1, 0, 2).reshape(128, -1)
        .astype(np.float16))

    s_idx = np.arange(128)[:, None]
    q_idx = np.arange(512)[None, :]
    masks_np = np.concatenate(
        [(q_idx >= 128 * r + s_idx).astype(np.float16) for r in range(4)],
        axis=1)  # [128, 2048]
    masks_np = np.ascontiguousarray(masks_np)
    ident_np = np.eye(128, dtype=np.float16)

    in_maps = []
    for c in range(N_CORES):
        h0, h1 = 2 * c, 2 * c + 1
        kv = c // 2
        wqkv_full = np.concatenate([
            Wq[h0 * HD:(h0 + 1) * HD],
            Wq[h1 * HD:(h1 + 1) * HD],
            Wk[kv * HD:(kv + 1) * HD],
            Wv[kv * HD:(kv + 1) * HD],
        ], axis=0).T.astype(np.float16)  # [D, 512]
        # pack row-blocks of 128 side by side: [128, (D//128)*512]
        wqkv_np = np.ascontiguousarray(
            wqkv_full.reshape(D // 128, 128, 512).transpose(1, 0, 2)
            .reshape(128, -1))
        wp_np = np.ascontiguousarray(
            Wp[:, c * 256:(c + 1) * 256].T.astype(np.float16))  # [256, D]
        # rs = 1/sqrt(ssq*gsc + gbi) == gain/sqrt(ssq/HD + EPS)
        g3 = np.array([q_gain[h0], q_gain[h1], 1.0], dtype=np.float32)
        gsc_np = np.tile((1.0 / (HD * g3 * g3)).reshape(1, 3), (128, 1)).astype(np.float32)
        gbi_np = np.tile((EPS / (g3 * g3)).reshape(1, 3), (128, 1)).astype(np.float32)
        in_maps.append({
            "xt": xt_np,
            "wqkv": wqkv_np,
            "wp": wp_np,
            "cost": cost_np,
            "sint": sint_np,
            "gsc": np.ascontiguousarray(gsc_np),
            "gbi": np.ascontiguousarray(gbi_np),
            "masks": masks_np,
            "ident": ident_np,
        })
    return in_maps


def _assemble_output(results, T):
    BT = B * T
    NCH = BT // 1024
    full = np.concatenate(
        [np.concatenate([results[c][f"out{ch}"] for ch in range(NCH)], axis=1)
         for c in range(N_CORES)], axis=0)  # [2048, BT] = out.T, fp16
    return np.ascontiguousarray(
        full.reshape(D, B, T).transpose(1, 2, 0)).astype(np.float32)


def run_on_hw(x, Wq, Wk, Wv, Wp, q_gain, trace=False):
    from concourse.bass_utils import run_bass_kernel_spmd

    T = x.shape[1]
    if T not in _PROGRAM_CACHE:
        _PROGRAM_CACHE[T] = _build_program(T)
    nc = _PROGRAM_CACHE[T]
    in_maps = _prepare_in_maps(x, Wq, Wk, Wv, Wp, q_gain)
    res = run_bass_kernel_spmd(nc, in_maps, list(range(N_CORES)), trace=trace)
    out = _assemble_output(res.results, T)
    return out, res


def kernel(x, Wq, Wk, Wv, Wp, q_gain):
    out, _ = run_on_hw(x, Wq, Wk, Wv, Wp, q_gain, trace=False)
    return out
